# revision 71
# baseline (speedup 1.0000x reference)
"""GATv2 layer (KNN graph, K=32, self-loops) on 8 Trainium2 NeuronCores.

Strategy (data-parallel over target nodes, 1250 rows/core, software-pipelined
across 128-row tiles):
  - similarity s[i,j] = x_i.x_j - 0.5*|x_j|^2 (order-equivalent to -dist^2 per
    row).  s[i,i] is always the row max, so top-33 of s = {self} + 32 nearest
    neighbours: no diagonal masking.
  - PE computes s via fp16 hi/lo split (3 matmul passes, err ~3e-6); the
    -0.5|x_j|^2 term rides in as a K=2 fp16 seed matmul.  h_l/h_r transforms
    are single fp16 passes (err ~1e-3, fine for scores at 2e-2 tolerance).
  - top-33 selection per row: per-625-chunk top-8 (vector.max) + positions
    (max_index), then mark-and-extract rounds (max/match_replace) on the
    [128, 128] candidate array.  Winner indices come out as *values* of a
    masked index array, compacted by cumsum-rank + gpsimd local_scatter
    (no per-partition gather needed anywhere).
  - h_l rows (fp16, |att| folded, features permuted pos-block-first, plus a
    fused 0.25*p = 0.25*att.h_l column) live in DRAM; neighbour rows are
    fetched with chunked gpsimd.dma_gather (768B rows), and
    z = g + (h_r | 0.25*q) is formed in-place by DMA-engine accumulate adds.
  - u = relu(z) on the Act engine; e8 = att-sign tree-sum of u in fp16
    (pos block on gpsimd, neg block on DVE); e = 0.8*(e8 + zcol); softmax
    via Act exp (scale=0.8); weighted sum = fp16 tensor_scalar muls (4x DVE
    mode) + a k-tree; out = sum a_k z_k - h_r (softmax sums to 1).
  - risky rows (chunk overflow / rank-33 near-ties / mark-count mismatch) are
    flagged and recomputed exactly on the host (vectorized float64).
"""

import os
import sys

for _p in ("/opt/trn_rl_repo", os.path.expanduser("~/.axon_site/_ro/trn_rl_repo")):
    if os.path.isdir(_p) and _p not in sys.path:
        sys.path.insert(0, _p)

from contextlib import ExitStack

import numpy as np

import concourse.bass as bass
import concourse.tile as tile
from concourse import bacc, mybir

F16 = np.float16

CFG = dict(
    N=10000,      # nodes
    DIN=128,      # input features (must be 128: one PE contraction)
    DOUT=256,     # output features
    KNN=32,       # neighbours (excl. self)
    NCORES=8,
    SELW=625,     # selection chunk width (top-8 per chunk)
    JCH=500,      # similarity matmul free-dim chunk (PSUM bank: <=512 f32)
    GROW=384,     # gathered DRAM row length in fp16 (256 h_l + 1 p + 127 pad)
)

NEG = -1.0e30
f32 = mybir.dt.float32
f16 = mybir.dt.float16
i16 = mybir.dt.int16
u32 = mybir.dt.uint32
FT = mybir.ActivationFunctionType
ALU = mybir.AluOpType
AX = mybir.AxisListType
P = 128


def _tile_starts(rows):
    starts = list(range(0, rows - P + 1, P))
    if starts[-1] + P < rows:
        starts.append(rows - P)
    return starts


def _split16(a):
    hi = a.astype(F16)
    lo = (a - hi.astype(np.float32)).astype(F16)
    return hi, lo


def build_program(cfg):
    N, DOUT, KNN = cfg["N"], cfg["DOUT"], cfg["KNN"]
    SELW, JCH, GROW = cfg["SELW"], cfg["JCH"], cfg["GROW"]
    ROWS = N // cfg["NCORES"]
    SELC = (N + SELW - 1) // SELW
    assert N % SELW == 0
    K1 = KNN + 1
    K1p = K1 + (-K1) % 2                 # 34: xbar needs cols%16==0
    NI = K1p * P                         # dma_gather index count per tile
    NC16p = K1p * 8                      # wrapped index columns
    SR = (K1 + 7) // 8                   # selection rounds (5 for K1=33)
    CAND = SELC * 8
    starts = _tile_starts(ROWS)
    nhl = (N + P - 1) // P
    dpos = cfg.get("_dpos", DOUT)        # pos-sign feature count
    dneg = DOUT - dpos

    nc = bacc.Bacc("TRN2", debug=False,
                   dynamic_dma_scratch_size=16 * 1024)

    # ---- I/O ----
    def inp(name, shape, dt):
        return nc.dram_tensor(name, list(shape), dt, kind="ExternalInput")

    xhT = inp("xhT", (P, N), f16)
    xlT = inp("xlT", (P, N), f16)
    xhTo = inp("xhTo", (P, ROWS), f16)
    xlTo = inp("xlTo", (P, ROWS), f16)
    # seed2o: fp16 hi/lo of -0.5|x|^2 (cols :N) | ones2 (cols N:N+P)
    seed2o = inp("seed2o", (2, N + P), f16)
    # w16: wle (|att|-folded W_l + 0.25*wp col) | wr
    w16 = inp("w16", (P, 2 * DOUT + 1), f16)
    # f32c: brr | sgnr | invar | biasr | cw1
    f32c = inp("f32c", (P, 4 * DOUT + CAND), f32)
    out_d = nc.dram_tensor("out", [ROWS, DOUT], f32, kind="ExternalOutput")
    flg_d = nc.dram_tensor("flags", [ROWS, 1], f32, kind="ExternalOutput")

    jchunks = [(a, min(JCH, N - a)) for a in range(0, N, JCH)]

    with ExitStack() as ctx:
        tc = ctx.enter_context(tile.TileContext(nc))
        cpool = ctx.enter_context(tc.tile_pool(name="consts", bufs=1))
        dpool = ctx.enter_context(tc.tile_pool(name="dram", bufs=1, space="DRAM"))
        spool = ctx.enter_context(tc.tile_pool(name="stage", bufs=2, space="DRAM"))
        psum = ctx.enter_context(tc.tile_pool(name="psum", bufs=5, space="PSUM"))
        psum_hl = ctx.enter_context(tc.tile_pool(name="psum_hl", bufs=2, space="PSUM"))
        psum_h = ctx.enter_context(tc.tile_pool(name="psum_h", bufs=1, space="PSUM"))
        hpool = ctx.enter_context(tc.tile_pool(name="hl", bufs=2))
        sp = ctx.enter_context(tc.tile_pool(name="s", bufs=1))
        selp = ctx.enter_context(tc.tile_pool(name="sel", bufs=1))
        gp = ctx.enter_context(tc.tile_pool(name="g", bufs=2))
        up = ctx.enter_context(tc.tile_pool(name="u", bufs=2))
        smp = ctx.enter_context(tc.tile_pool(name="small", bufs=2))
        op = ctx.enter_context(tc.tile_pool(name="outs", bufs=2))

        # ---- load constants ----
        def load(t, nchunk=1):
            tl = cpool.tile(list(t.shape), t.dtype, tag=t.name)
            w = t.shape[1] // nchunk
            for c in range(nchunk):
                nc.sync.dma_start(tl[:, c * w:(c + 1) * w],
                                  t.ap()[:, c * w:(c + 1) * w])
            return tl

        w16_s = load(w16)
        xhTo_s, xlTo_s = load(xhTo), load(xlTo)
        s2o = load(seed2o)
        xhT_s = cpool.tile(list(xhT.shape), xhT.dtype, tag="xhT")
        xlT_s = cpool.tile(list(xlT.shape), xlT.dtype, tag="xlT")
        wch = N // 8
        for c in range(8):
            sl = slice(c * wch, (c + 1) * wch)
            nc.sync.dma_start(xhT_s[:, sl], xhT.ap()[:, sl])
            nc.sync.dma_start(xlT_s[:, sl], xlT.ap()[:, sl])
        f32c_s = load(f32c)
        wle_s = w16_s[:, :DOUT + 1]
        wr_s = w16_s[:, DOUT + 1:]
        seed2_s = s2o[:, :N]
        ones2_s = s2o[:, N:]
        brr_s = f32c_s[:, 0:DOUT]
        sgnr_s = f32c_s[:, DOUT:2 * DOUT]
        invar_s = f32c_s[:, 2 * DOUT:3 * DOUT]
        biasr_s = f32c_s[:, 3 * DOUT:4 * DOUT]
        cw1_s = f32c_s[:, 4 * DOUT:]
        zeros_s = cpool.tile([P, CAND], f32, tag="zeros")
        nc.gpsimd.memset(zeros_s[:], 0.0)

        # ---- phase B: h_l (+0.25p column) fp16 for all nodes -> DRAM ----
        hl_d = dpool.tile([N, GROW], f16)

        def phase_b():
            for i in range(nhl):
                w_ = min(P, N - i * P)
                ps = psum_hl.tile([P, DOUT + 1], f32, tag="hlp")
                lo = i * P
                nc.tensor.matmul(ps[:w_], xhT_s[:, lo:lo + w_], wle_s,
                                 start=True, stop=True)
                hb = hpool.tile([P, DOUT + 1], f16, tag="hb")
                # split psum->fp16 copies across Act/DVE
                if i % 2 == 0:
                    nc.scalar.activation(hb[:w_], ps[:w_], FT.Copy)
                else:
                    nc.vector.tensor_copy(hb[:w_], ps[:w_])
                nc.sync.dma_start(hl_d[lo:lo + w_, :DOUT + 1], hb[:w_])

        # ---- phase C: software-pipelined per 128-row tile ----
        def sel_part(ts_, relu_cb=None, last=False):
            d = {}
            # h_r for this tile (+ q = att.h_r)
            pr = psum_h.tile([P, DOUT], f32, tag="hrp")
            nc.tensor.matmul(pr[:], xhTo_s[:, ts_:ts_ + P], wr_s,
                             start=True, stop=True)
            hrq = smp.tile([P, DOUT + 1], f16, tag="hrq")
            nc.vector.tensor_add(hrq[:, :DOUT], pr[:], brr_s)
            tscr = smp.tile([P, DOUT], f32, tag="tscr", bufs=1)
            q02 = smp.tile([P, 1], f32, tag="q02")
            nc.vector.scalar_tensor_tensor(
                tscr[:], hrq[:, :DOUT], 1.0, sgnr_s,
                op0=ALU.mult, op1=ALU.mult, accum_out=q02[:])
            nc.vector.tensor_scalar_mul(q02[:], q02[:], 0.25)
            nc.vector.tensor_copy(hrq[:, DOUT:], q02[:])
            d.update(hrq=hrq, q02=q02)

            # similarity row-block s = x_i.x_j - 0.5|x_j|^2
            s_sb = sp.tile([P, N], f32, tag="s")
            for ci, (a, w_) in enumerate(jchunks):
                ps = psum.tile([P, w_], f32, tag="sp")
                nc.tensor.matmul(ps[:], xhTo_s[:, ts_:ts_ + P], xhT_s[:, a:a + w_], start=True, stop=False)
                nc.tensor.matmul(ps[:], xhTo_s[:, ts_:ts_ + P], xlT_s[:, a:a + w_], start=False, stop=False)
                nc.tensor.matmul(ps[:], xlTo_s[:, ts_:ts_ + P], xhT_s[:, a:a + w_], start=False, stop=False)
                nc.tensor.matmul(ps[:], ones2_s, seed2_s[:, a:a + w_], start=False, stop=True)
                nc.scalar.activation(s_sb[:, a:a + w_], ps[:], FT.Copy)
                if ci == 3 and relu_cb is not None:
                    relu_cb()

            # --- selection: per-chunk top-8 + indices ---
            v8 = selp.tile([P, CAND], f32, tag="v8")
            l8 = selp.tile([P, CAND], u32, tag="l8")
            for c in range(SELC):
                nc.vector.max(v8[:, 8 * c:8 * c + 8], s_sb[:, SELW * c:SELW * (c + 1)])
                nc.vector.max_index(l8[:, 8 * c:8 * c + 8], v8[:, 8 * c:8 * c + 8],
                                    s_sb[:, SELW * c:SELW * (c + 1)])
            glp1 = selp.tile([P, CAND], f32, tag="glp1")
            nc.vector.tensor_copy(glp1[:], l8[:])
            nc.vector.tensor_add(glp1[:], glp1[:], cw1_s)  # global_idx + 1

            # --- rounds on values: mark top-33 with NEG ---
            candA = selp.tile([P, CAND], f32, tag="candA")
            candB = selp.tile([P, CAND], f32, tag="candB")
            cur = v8
            for rr in range(SR - 1):
                m8 = smp.tile([P, 8], f32, tag=f"m8_{rr % 2}")
                nc.vector.max(m8[:], cur[:])
                nxt = candA if rr % 2 == 0 else candB
                nc.vector.match_replace(nxt[:], m8[:], cur[:], NEG)
                cur = nxt
            nlast = K1 - 8 * (SR - 1)          # 1 for K1=33
            m5 = smp.tile([P, 8], f32, tag="m5")
            nc.vector.max(m5[:], cur[:])
            vx8 = smp.tile([P, 8], f32, tag="vx8")
            nc.vector.tensor_copy(vx8[:], m5[:, nlast - 1:nlast].broadcast_to((P, 8)))
            fin = candB if cur is candA else candA
            nc.vector.match_replace(fin[:], vx8[:], cur[:], NEG)

            # --- mask -> masked global indices -> extract as values ---
            mask = selp.tile([P, CAND], f32, tag="mask")
            fcnt = smp.tile([P, 1], f32, tag="fcnt")
            nc.vector.tensor_scalar(mask[:], fin[:], -1.0e29, None, op0=ALU.is_le)
            nc.vector.tensor_reduce(fcnt[:], mask[:], axis=AX.X, op=ALU.add)
            midxA = selp.tile([P, CAND], f32, tag="midxA")
            nc.vector.tensor_mul(midxA[:], glp1[:], mask[:])
            midxB = selp.tile([P, CAND], f32, tag="midxB")
            nc.vector.tensor_scalar_add(midxB[:], midxA[:], -1.0)

            # --- extract the K1 marked indices compactly: rank = cumsum(mask),
            #     then gpsimd local_scatter(data=idx, idxs=rank-1) ---
            rank = selp.tile([P, CAND], f32, tag="rank")
            nc.vector.tensor_tensor_scan(rank[:], mask[:], zeros_s[:], 0.0,
                                         op0=ALU.add, op1=ALU.add)
            sidx = selp.tile([P, CAND], f32, tag="sidx")
            nc.vector.scalar_tensor_tensor(sidx[:], rank[:], 1.0, mask[:],
                                           op0=ALU.mult, op1=ALU.mult)
            nc.vector.tensor_scalar_add(sidx[:], sidx[:], -1.0)
            nc.vector.tensor_scalar_min(sidx[:], sidx[:], float(K1p - 1))
            sidx16 = selp.tile([P, CAND], i16, tag="sidx16")
            data16 = selp.tile([P, CAND], i16, tag="data16")
            nc.vector.tensor_copy(sidx16[:], sidx[:])
            nc.vector.tensor_copy(data16[:], midxB[:])
            tc_i = smp.tile([P, K1p], i16, tag="tc_i", bufs=1)
            nc.gpsimd.local_scatter(tc_i[:], data16[:], sidx16[:],
                                    channels=P, num_elems=K1p, num_idxs=CAND)
            stg = spool.tile([P * K1p], i16, tag="stg")
            nc.sync.dma_start(stg[:].rearrange("(p c) -> p c", p=P), tc_i[:])
            idx16 = smp.tile([P, NC16p], i16, tag="idx16")
            srcv = stg[:].rearrange("(g p k) -> p k g", g=8, p=16)
            nc.sync.dma_start(idx16[0:16, :], srcv)
            try:
                nc.sync.dma_start(
                    idx16[16:, :].rearrange("(r p) c -> r p c", r=7),
                    idx16[0:16, :].broadcast_to((7, 16, NC16p)))
            except Exception:
                for rr in range(1, 8):
                    nc.sync.dma_start(idx16[16 * rr:16 * (rr + 1), :], idx16[0:16, :])

            # --- risky-row flags: (a) possible chunk overflow, (b) tiny
            #     rank-33/34 margin, (c) mark-count != K1.  Host recomputes. ---
            flg = smp.tile([P, 1], f32, tag="flg")
            f40 = smp.tile([P, SELC], f32, tag="f40")
            v8l = v8[:].rearrange("p (c e) -> p c e", e=8)[:, :, 7]
            nc.gpsimd.tensor_scalar(f40[:], v8l, m5[:, 0:1], None, op0=ALU.is_ge)
            nc.vector.tensor_reduce(flg[:], f40[:], axis=AX.X, op=ALU.max)
            fm = smp.tile([P, 1], f32, tag="fm")
            nc.gpsimd.tensor_sub(fm[:], m5[:, 0:1], m5[:, 1:2])
            nc.gpsimd.tensor_scalar(fm[:], fm[:], 5.0e-4, None, op0=ALU.is_lt)
            nc.gpsimd.tensor_add(flg[:], flg[:], fm[:])
            fc = smp.tile([P, 1], f32, tag="fc")
            nc.gpsimd.tensor_scalar(fc[:], fcnt[:], float(K1), None, op0=ALU.subtract)
            nc.gpsimd.tensor_scalar(fc[:], fc[:], 0.0, None, op0=ALU.not_equal)
            nc.gpsimd.tensor_add(flg[:], flg[:], fc[:])
            nc.sync.dma_start(flg_d.ap()[ts_:ts_ + P, :], flg[:])


            d.update(ts=ts_, idx16=idx16)
            return d

        def gather_part(d, last=False):
            # gathers + z-accum: g rows of (h_l | 0.25 p), then
            # g[:, :, :257] += (h_r | 0.25 q) via DMA-engine accumulate
            g = gp.tile([P, K1p, GROW], f16, tag="g")
            hrq, idx16 = d["hrq"], d["idx16"]
            for c0, c1 in ((0, 8), (8, 16), (16, 24), (24, 32), (32, K1p)):
                ni = (c1 - c0) * P
                nc.gpsimd.dma_gather(g[:, c0:c1, :], hl_d[:],
                                     idx16[:, c0 * 8:c1 * 8],
                                     num_idxs=ni, num_idxs_reg=ni,
                                     elem_size=GROW)
                if not last:
                    ca, cb = c0, min(c1, K1)
                    nc.gpsimd.dma_start(
                        g[:, ca:cb, :DOUT + 1],
                        hrq[:].rearrange("p (o d) -> p o d", o=1)
                        .broadcast_to((P, cb - ca, DOUT + 1)),
                        accum_op=ALU.add)
            u = up.tile([P, K1, DOUT], f16, tag="u")
            d.update(g=g, u=u)

        def relu_part(d, last=False):
            u, g, hrq = d["u"], d["g"], d["hrq"]
            if last:
                # z into u on DVE, then relu u -> g's score region
                for c0, c1 in ((0, 8), (8, 16), (16, 24), (24, 32), (32, K1)):
                    nc.vector.tensor_add(
                        u[:, c0:c1, :], g[:, c0:c1, :DOUT],
                        hrq[:, :DOUT].rearrange("p (o d) -> p o d", o=1)
                        .broadcast_to((P, c1 - c0, DOUT)))
                nc.vector.tensor_scalar_max(g[:, :K1, :DOUT], u[:], 0.0)
            else:
                nc.scalar.activation(u[:], g[:, :K1, :DOUT], FT.Relu)

        def score_part(d):
            g, u, hrq, ts_ = d["g"], d["u"], d["hrq"], d["ts"]
            last = bool(d.get("last"))
            rt = g[:, :K1, :DOUT] if last else u[:]     # relu'd tensor
            zt = u[:] if last else g[:, :K1, :DOUT]     # z tensor (ws source)
            wt = g[:, :K1, :DOUT] if last else u[:]     # ws scratch

            def tree(off, w, eng):
                while w > 1:
                    a_ = w // 2
                    nw = w - a_
                    eng.tensor_add(rt[:, :, off:off + a_],
                                   rt[:, :, off:off + a_],
                                   rt[:, :, off + nw:off + w])
                    w = nw

            ee = smp.tile([P, K1], f16, tag="ee")
            if 0 < dpos < DOUT:
                tree(0, dpos, nc.vector if last else nc.gpsimd)
                tree(dpos, dneg, nc.vector)
                nc.vector.tensor_sub(ee[:], rt[:, :, 0], rt[:, :, dpos])
            else:
                tree(0, DOUT, nc.vector)
                if dpos == 0:
                    nc.vector.tensor_scalar_mul(ee[:], rt[:, :, 0], -1.0)
                else:
                    nc.vector.tensor_copy(ee[:], rt[:, :, 0])
            # e/0.8 = e8 + 0.25*p + 0.25*q;  softmax via Act exp, scale=0.8
            nc.vector.tensor_add(ee[:], ee[:], g[:, :K1, DOUT])
            if last:
                nc.vector.tensor_scalar_add(ee[:], ee[:], d["q02"][:])
            mx = smp.tile([P, 1], f32, tag="mx")
            nc.vector.reduce_max(mx[:], ee[:], axis=AX.X)
            nc.vector.tensor_scalar_mul(mx[:], mx[:], -0.8)
            ex = smp.tile([P, K1], f32, tag="ex")
            nc.scalar.activation(ex[:], ee[:], FT.Exp, bias=mx[:], scale=0.8)
            sm = smp.tile([P, 1], f32, tag="sm")
            nc.vector.reduce_sum(sm[:], ex[:], axis=AX.X)
            nc.vector.reciprocal(sm[:], sm[:])
            al = smp.tile([P, K1], f32, tag="al")
            nc.vector.tensor_scalar_mul(al[:], ex[:], sm[:])

            # --- weighted sum: alpha-muls into ws scratch, k-tree ---
            for k in range(K1):
                nc.vector.tensor_scalar_mul(wt[:, k, :], zt[:, k, :],
                                            al[:, k:k + 1])
            kw = K1
            while kw > 1:
                a_ = kw // 2
                nw = kw - a_
                nc.vector.tensor_add(wt[:, 0:a_, :], wt[:, 0:a_, :],
                                     wt[:, nw:kw, :])
                kw = nw
            ob = op.tile([P, DOUT], f32, tag="ob")
            nc.vector.tensor_sub(ob[:], wt[:, 0, :], hrq[:, :DOUT])
            nc.vector.tensor_mul(ob[:], ob[:], invar_s)
            nc.vector.tensor_add(ob[:], ob[:], biasr_s)
            nc.sync.dma_start(out_d.ap()[ts_:ts_ + P, :], ob[:])

        prev = None
        for ii, ts_ in enumerate(starts):
            _cb = (lambda p=prev: relu_part(p)) if prev is not None else None
            last_t = (ii == len(starts) - 1)
            d = sel_part(ts_, relu_cb=_cb, last=last_t)
            gather_part(d, last=last_t)
            if ii == 0:
                phase_b()
            if prev is not None:
                score_part(prev)
            prev = d
        prev["last"] = True
        relu_part(prev, last=True)
        score_part(prev)

    nc.compile()
    return nc


def host_prep(x, W_l, b_l, W_r, b_r, att, bias, cfg):
    """Build the per-core input maps (numpy only; cheap O(N*D) work)."""
    N, DOUT = cfg["N"], cfg["DOUT"]
    ROWS = N // cfg["NCORES"]
    SELC = N // cfg["SELW"]
    CAND = SELC * 8

    x = np.asarray(x, np.float32)
    xh, xl = _split16(x)
    xhT = np.ascontiguousarray(xh.T)
    xlT = np.ascontiguousarray(xl.T)
    sq = (x.astype(np.float64) ** 2).sum(1)
    sv = (-0.5 * sq).astype(np.float32)
    shi = sv.astype(F16)
    slo = (sv - shi.astype(np.float32)).astype(F16)
    seed2 = np.stack([shi, slo], 0)
    seed2o = np.concatenate([seed2, np.ones((2, P), F16)], 1)

    att = np.asarray(att, np.float32)
    # permute output features: att>0 block first; fold |att| into weights.
    perm = np.argsort(att <= 0, kind="stable")
    aperm = att[perm]
    aabs = np.maximum(np.abs(aperm), 1e-30)
    wp = (W_l.astype(np.float64) @ att.astype(np.float64)).astype(np.float32)
    wle = np.concatenate([W_l[:, perm] * aabs[None, :], 0.25 * wp[:, None]],
                         1).astype(F16)
    wr = (np.asarray(W_r, np.float32)[:, perm] * aabs[None, :]).astype(F16)
    bl = np.asarray(b_l, np.float32)
    # z-bias: (b_l+b_r) |att|-scaled rides on h_r; p-column omits att.b_l but
    # q (= att.h_r incl. b_l+b_r) supplies it, so p_j + q_n is exact.
    brr = np.tile(((np.asarray(b_r, np.float32) + bl)[perm]
                   * aabs)[None, :], (P, 1))
    sgnr = np.tile(np.sign(aperm)[None, :], (P, 1)).astype(np.float32)
    invar = np.tile((1.0 / aabs)[None, :], (P, 1)).astype(np.float32)
    biasr = np.tile((np.asarray(bias, np.float32)
                     + np.asarray(b_l, np.float32))[perm][None, :], (P, 1))
    cwrow = (np.arange(CAND) // 8 * cfg["SELW"] + 1).astype(np.float32)
    cw1 = np.tile(cwrow[None, :], (P, 1))
    w16 = np.concatenate([wle, wr], 1)
    f32c = np.concatenate([brr, sgnr, invar, biasr, cw1], 1).astype(np.float32)

    shared = dict(xhT=xhT, xlT=xlT, seed2o=seed2o, w16=w16, f32c=f32c)
    host_prep.last_perm = perm
    host_prep.last_dpos = int((aperm > 0).sum())
    in_maps = []
    for c in range(cfg["NCORES"]):
        R = c * ROWS
        m = dict(shared)
        m["xhTo"] = np.ascontiguousarray(xhT[:, R:R + ROWS])
        m["xlTo"] = np.ascontiguousarray(xlT[:, R:R + ROWS])
        in_maps.append(m)
    return in_maps


_PROG_CACHE = {}


def _get_program(dpos):
    if dpos not in _PROG_CACHE:
        cfg = dict(CFG)
        cfg["_dpos"] = dpos
        _PROG_CACHE[dpos] = build_program(cfg)
    return _PROG_CACHE[dpos]


def kernel(x, W_l, b_l, W_r, b_r, att, bias, _trace=False):
    from concourse import bass_utils

    cfg = CFG
    in_maps = host_prep(x, W_l, b_l, W_r, b_r, att, bias, cfg)
    perm = host_prep.last_perm
    nc = _get_program(host_prep.last_dpos)
    try:
        res = bass_utils.run_bass_kernel_spmd(
            nc, in_maps, core_ids=list(range(cfg["NCORES"])), trace=_trace)
    except ModuleNotFoundError:
        res = bass_utils.run_bass_kernel_spmd(
            nc, in_maps, core_ids=list(range(cfg["NCORES"])), trace=False)
    outp = np.concatenate([r["out"] for r in res.results], 0)
    out = np.empty_like(outp)
    out[:, perm] = outp
    kernel.last_exec_time_ns = res.exec_time_ns
    flags = np.concatenate([r["flags"][:, 0] for r in res.results], 0)
    rows = np.where(flags != 0.0)[0]
    if rows.size:
        _patch_rows(out, rows, x, W_l, b_l, W_r, b_r, att, bias, cfg)
    return out.astype(np.float32)


def _patch_rows(out, rows, x, W_l, b_l, W_r, b_r, att, bias, cfg):
    """Exact (float64) recompute of flagged rows (near-ties / rare overflow),
    vectorized over the flagged set."""
    K = cfg["KNN"]
    x64 = np.asarray(x, np.float64)
    sq = (x64 * x64).sum(1)
    h_l = x64 @ np.asarray(W_l, np.float64) + np.asarray(b_l, np.float64)
    att64 = np.asarray(att, np.float64)
    W_r64 = np.asarray(W_r, np.float64)
    rows = np.asarray(rows)
    R = len(rows)
    d = sq[None, :] + sq[rows, None] - 2.0 * (x64[rows] @ x64.T)  # [R, N]
    d[np.arange(R), rows] = np.inf
    nbr = np.argpartition(d, K, axis=1)[:, :K]                    # [R, K]
    src = np.concatenate([nbr, rows[:, None]], 1)                 # [R, K+1]
    h_r = x64[rows] @ W_r64 + np.asarray(b_r, np.float64)         # [R, D]
    z = h_l[src] + h_r[:, None, :]                                # [R, K+1, D]
    lr = np.where(z > 0, z, 0.2 * z)
    e = lr @ att64                                                # [R, K+1]
    e = e - e.max(1, keepdims=True)
    a = np.exp(e)
    a /= a.sum(1, keepdims=True)
    o = np.einsum('rk,rkd->rd', a, h_l[src]) + np.asarray(bias, np.float64)
    out[rows] = o.astype(np.float32)


# revision 74
# speedup vs baseline: 1.0150x; 1.0150x over previous
"""GATv2 layer (KNN graph, K=32, self-loops) on 8 Trainium2 NeuronCores.

Strategy (data-parallel over target nodes, 1250 rows/core, software-pipelined
across 128-row tiles):
  - similarity s[i,j] = x_i.x_j - 0.5*|x_j|^2 (order-equivalent to -dist^2 per
    row).  s[i,i] is always the row max, so top-33 of s = {self} + 32 nearest
    neighbours: no diagonal masking.
  - PE computes s via fp16 hi/lo split (3 matmul passes, err ~3e-6); the
    -0.5|x_j|^2 term rides in as a K=2 fp16 seed matmul.  h_l/h_r transforms
    are single fp16 passes (err ~1e-3, fine for scores at 2e-2 tolerance).
  - top-33 selection per row: per-625-chunk top-8 (vector.max) + positions
    (max_index), then mark-and-extract rounds (max/match_replace) on the
    [128, 128] candidate array.  Winner indices come out as *values* of a
    masked index array, compacted by cumsum-rank + gpsimd local_scatter
    (no per-partition gather needed anywhere).
  - h_l rows (fp16, |att| folded, features permuted pos-block-first, plus a
    fused 0.25*p = 0.25*att.h_l column) live in DRAM; neighbour rows are
    fetched with chunked gpsimd.dma_gather (768B rows), and
    z = g + (h_r | 0.25*q) is formed in-place by DMA-engine accumulate adds.
  - u = relu(z) on the Act engine; e8 = att-sign tree-sum of u in fp16
    (pos block on gpsimd, neg block on DVE); e = 0.8*(e8 + zcol); softmax
    via Act exp (scale=0.8); weighted sum = fp16 tensor_scalar muls (4x DVE
    mode) + a k-tree; out = sum a_k z_k - h_r (softmax sums to 1).
  - risky rows (chunk overflow / rank-33 near-ties / mark-count mismatch) are
    flagged and recomputed exactly on the host (vectorized float64).
"""

import os
import sys

for _p in ("/opt/trn_rl_repo", os.path.expanduser("~/.axon_site/_ro/trn_rl_repo")):
    if os.path.isdir(_p) and _p not in sys.path:
        sys.path.insert(0, _p)

from contextlib import ExitStack

import numpy as np

import concourse.bass as bass
import concourse.tile as tile
from concourse import bacc, mybir

F16 = np.float16

CFG = dict(
    N=10000,      # nodes
    DIN=128,      # input features (must be 128: one PE contraction)
    DOUT=256,     # output features
    KNN=32,       # neighbours (excl. self)
    NCORES=8,
    SELW=625,     # selection chunk width (top-8 per chunk)
    JCH=500,      # similarity matmul free-dim chunk (PSUM bank: <=512 f32)
    GROW=384,     # gathered DRAM row length in fp16 (256 h_l + 1 p + 127 pad)
)

NEG = -1.0e30
f32 = mybir.dt.float32
f16 = mybir.dt.float16
i16 = mybir.dt.int16
u32 = mybir.dt.uint32
FT = mybir.ActivationFunctionType
ALU = mybir.AluOpType
AX = mybir.AxisListType
P = 128


def _tile_starts(rows):
    starts = list(range(0, rows - P + 1, P))
    if starts[-1] + P < rows:
        starts.append(rows - P)
    return starts


def _split16(a):
    hi = a.astype(F16)
    lo = (a - hi.astype(np.float32)).astype(F16)
    return hi, lo


def build_program(cfg):
    N, DOUT, KNN = cfg["N"], cfg["DOUT"], cfg["KNN"]
    SELW, JCH, GROW = cfg["SELW"], cfg["JCH"], cfg["GROW"]
    ROWS = N // cfg["NCORES"]
    SELC = (N + SELW - 1) // SELW
    assert N % SELW == 0
    K1 = KNN + 1
    K1p = K1 + (-K1) % 2                 # 34: xbar needs cols%16==0
    NI = K1p * P                         # dma_gather index count per tile
    NC16p = K1p * 8                      # wrapped index columns
    SR = (K1 + 7) // 8                   # selection rounds (5 for K1=33)
    CAND = SELC * 8
    starts = _tile_starts(ROWS)
    nhl = (N + P - 1) // P
    dpos = cfg.get("_dpos", DOUT)        # pos-sign feature count
    dneg = DOUT - dpos

    nc = bacc.Bacc("TRN2", debug=False,
                   dynamic_dma_scratch_size=16 * 1024)

    # ---- I/O ----
    def inp(name, shape, dt):
        return nc.dram_tensor(name, list(shape), dt, kind="ExternalInput")

    xhT = inp("xhT", (P, N), f16)
    xlT = inp("xlT", (P, N), f16)
    xhTo = inp("xhTo", (P, ROWS), f16)
    xlTo = inp("xlTo", (P, ROWS), f16)
    # seed2o: fp16 hi/lo of -0.5|x|^2 (cols :N) | ones2 (cols N:N+P)
    seed2o = inp("seed2o", (2, N + P), f16)
    # w16: wle (|att|-folded W_l + 0.25*wp col) | wr
    w16 = inp("w16", (P, 2 * DOUT + 1), f16)
    # f32c: brr | sgnr | invar | biasr | cw1
    f32c = inp("f32c", (P, 4 * DOUT + CAND), f32)
    out_d = nc.dram_tensor("out", [ROWS, DOUT], f32, kind="ExternalOutput")
    flg_d = nc.dram_tensor("flags", [ROWS, 1], f32, kind="ExternalOutput")

    jchunks = [(a, min(JCH, N - a)) for a in range(0, N, JCH)]

    with ExitStack() as ctx:
        tc = ctx.enter_context(tile.TileContext(nc))
        cpool = ctx.enter_context(tc.tile_pool(name="consts", bufs=1))
        dpool = ctx.enter_context(tc.tile_pool(name="dram", bufs=1, space="DRAM"))
        spool = ctx.enter_context(tc.tile_pool(name="stage", bufs=2, space="DRAM"))
        psum = ctx.enter_context(tc.tile_pool(name="psum", bufs=5, space="PSUM"))
        psum_hl = ctx.enter_context(tc.tile_pool(name="psum_hl", bufs=2, space="PSUM"))
        psum_h = ctx.enter_context(tc.tile_pool(name="psum_h", bufs=1, space="PSUM"))
        hpool = ctx.enter_context(tc.tile_pool(name="hl", bufs=2))
        sp = ctx.enter_context(tc.tile_pool(name="s", bufs=1))
        selp = ctx.enter_context(tc.tile_pool(name="sel", bufs=1))
        gp = ctx.enter_context(tc.tile_pool(name="g", bufs=2))
        up = ctx.enter_context(tc.tile_pool(name="u", bufs=2))
        smp = ctx.enter_context(tc.tile_pool(name="small", bufs=2))
        op = ctx.enter_context(tc.tile_pool(name="outs", bufs=2))

        # ---- load constants ----
        def load(t, nchunk=1):
            tl = cpool.tile(list(t.shape), t.dtype, tag=t.name)
            w = t.shape[1] // nchunk
            for c in range(nchunk):
                nc.sync.dma_start(tl[:, c * w:(c + 1) * w],
                                  t.ap()[:, c * w:(c + 1) * w])
            return tl

        w16_s = load(w16)
        xhTo_s, xlTo_s = load(xhTo), load(xlTo)
        s2o = load(seed2o)
        xhT_s = cpool.tile(list(xhT.shape), xhT.dtype, tag="xhT")
        xlT_s = cpool.tile(list(xlT.shape), xlT.dtype, tag="xlT")
        wch = N // 8
        for c in range(8):
            sl = slice(c * wch, (c + 1) * wch)
            nc.sync.dma_start(xhT_s[:, sl], xhT.ap()[:, sl])
            nc.sync.dma_start(xlT_s[:, sl], xlT.ap()[:, sl])
        f32c_s = load(f32c)
        wle_s = w16_s[:, :DOUT + 1]
        wr_s = w16_s[:, DOUT + 1:]
        seed2_s = s2o[:, :N]
        ones2_s = s2o[:, N:]
        brr_s = f32c_s[:, 0:DOUT]
        sgnr_s = f32c_s[:, DOUT:2 * DOUT]
        invar_s = f32c_s[:, 2 * DOUT:3 * DOUT]
        biasr_s = f32c_s[:, 3 * DOUT:4 * DOUT]
        cw1_s = f32c_s[:, 4 * DOUT:]
        zeros_s = cpool.tile([P, CAND], f32, tag="zeros")
        nc.gpsimd.memset(zeros_s[:], 0.0)

        # ---- phase B: h_l (+0.25p column) fp16 for all nodes -> DRAM ----
        hl_d = dpool.tile([N, GROW], f16)

        def phase_b(i0=0, i1=None):
            for i in range(i0, nhl if i1 is None else i1):
                w_ = min(P, N - i * P)
                ps = psum_hl.tile([P, DOUT + 1], f32, tag="hlp")
                lo = i * P
                nc.tensor.matmul(ps[:w_], xhT_s[:, lo:lo + w_], wle_s,
                                 start=True, stop=True)
                hb = hpool.tile([P, DOUT + 1], f16, tag="hb")
                # split psum->fp16 copies across Act/DVE
                nc.scalar.activation(hb[:w_], ps[:w_], FT.Copy)
                nc.sync.dma_start(hl_d[lo:lo + w_, :DOUT + 1], hb[:w_])

        # ---- phase C: software-pipelined per 128-row tile ----
        def sel_part(ts_, relu_cb=None, last=False):
            d = {}
            # h_r for this tile (+ q = att.h_r)
            pr = psum_h.tile([P, DOUT], f32, tag="hrp")
            nc.tensor.matmul(pr[:], xhTo_s[:, ts_:ts_ + P], wr_s,
                             start=True, stop=True)
            hrq = smp.tile([P, DOUT + 1], f16, tag="hrq")
            nc.vector.tensor_add(hrq[:, :DOUT], pr[:], brr_s)
            tscr = smp.tile([P, DOUT], f32, tag="tscr", bufs=1)
            q02 = smp.tile([P, 1], f32, tag="q02")
            nc.vector.scalar_tensor_tensor(
                tscr[:], hrq[:, :DOUT], 1.0, sgnr_s,
                op0=ALU.mult, op1=ALU.mult, accum_out=q02[:])
            nc.vector.tensor_scalar_mul(q02[:], q02[:], 0.25)
            nc.vector.tensor_copy(hrq[:, DOUT:], q02[:])
            d.update(hrq=hrq, q02=q02)

            # similarity row-block s = x_i.x_j - 0.5|x_j|^2
            s_sb = sp.tile([P, N], f32, tag="s")
            for ci, (a, w_) in enumerate(jchunks):
                ps = psum.tile([P, w_], f32, tag="sp")
                nc.tensor.matmul(ps[:], xhTo_s[:, ts_:ts_ + P], xhT_s[:, a:a + w_], start=True, stop=False)
                nc.tensor.matmul(ps[:], xhTo_s[:, ts_:ts_ + P], xlT_s[:, a:a + w_], start=False, stop=False)
                nc.tensor.matmul(ps[:], xlTo_s[:, ts_:ts_ + P], xhT_s[:, a:a + w_], start=False, stop=False)
                nc.tensor.matmul(ps[:], ones2_s, seed2_s[:, a:a + w_], start=False, stop=True)
                nc.scalar.activation(s_sb[:, a:a + w_], ps[:], FT.Copy)
                if ci == 3 and relu_cb is not None:
                    relu_cb()

            # --- selection: per-chunk top-8 + indices ---
            v8 = selp.tile([P, CAND], f32, tag="v8")
            l8 = selp.tile([P, CAND], u32, tag="l8")
            for c in range(SELC):
                nc.vector.max(v8[:, 8 * c:8 * c + 8], s_sb[:, SELW * c:SELW * (c + 1)])
                nc.vector.max_index(l8[:, 8 * c:8 * c + 8], v8[:, 8 * c:8 * c + 8],
                                    s_sb[:, SELW * c:SELW * (c + 1)])
            glp1 = selp.tile([P, CAND], f32, tag="glp1")
            nc.vector.tensor_copy(glp1[:], l8[:])
            nc.vector.tensor_add(glp1[:], glp1[:], cw1_s)  # global_idx + 1

            # --- rounds on values: mark top-33 with NEG ---
            candA = selp.tile([P, CAND], f32, tag="candA")
            candB = selp.tile([P, CAND], f32, tag="candB")
            cur = v8
            for rr in range(SR - 1):
                m8 = smp.tile([P, 8], f32, tag=f"m8_{rr % 2}")
                nc.vector.max(m8[:], cur[:])
                nxt = candA if rr % 2 == 0 else candB
                nc.vector.match_replace(nxt[:], m8[:], cur[:], NEG)
                cur = nxt
            nlast = K1 - 8 * (SR - 1)          # 1 for K1=33
            m5 = smp.tile([P, 8], f32, tag="m5")
            nc.vector.max(m5[:], cur[:])
            vx8 = smp.tile([P, 8], f32, tag="vx8")
            nc.vector.tensor_copy(vx8[:], m5[:, nlast - 1:nlast].broadcast_to((P, 8)))
            fin = candB if cur is candA else candA
            nc.vector.match_replace(fin[:], vx8[:], cur[:], NEG)

            # --- mask -> masked global indices -> extract as values ---
            mask = selp.tile([P, CAND], f32, tag="mask")
            fcnt = smp.tile([P, 1], f32, tag="fcnt")
            nc.vector.tensor_scalar(mask[:], fin[:], -1.0e29, None, op0=ALU.is_le)
            nc.vector.tensor_reduce(fcnt[:], mask[:], axis=AX.X, op=ALU.add)
            midxA = selp.tile([P, CAND], f32, tag="midxA")
            nc.vector.tensor_mul(midxA[:], glp1[:], mask[:])
            midxB = selp.tile([P, CAND], f32, tag="midxB")
            nc.vector.tensor_scalar_add(midxB[:], midxA[:], -1.0)

            # --- extract the K1 marked indices compactly: rank = cumsum(mask),
            #     then gpsimd local_scatter(data=idx, idxs=rank-1) ---
            rank = selp.tile([P, CAND], f32, tag="rank")
            nc.vector.tensor_tensor_scan(rank[:], mask[:], zeros_s[:], 0.0,
                                         op0=ALU.add, op1=ALU.add)
            sidx = selp.tile([P, CAND], f32, tag="sidx")
            nc.vector.scalar_tensor_tensor(sidx[:], rank[:], 1.0, mask[:],
                                           op0=ALU.mult, op1=ALU.mult)
            nc.vector.tensor_scalar_add(sidx[:], sidx[:], -1.0)
            nc.vector.tensor_scalar_min(sidx[:], sidx[:], float(K1p - 1))
            sidx16 = selp.tile([P, CAND], i16, tag="sidx16")
            data16 = selp.tile([P, CAND], i16, tag="data16")
            nc.vector.tensor_copy(sidx16[:], sidx[:])
            nc.vector.tensor_copy(data16[:], midxB[:])
            tc_i = smp.tile([P, K1p], i16, tag="tc_i", bufs=1)
            nc.gpsimd.local_scatter(tc_i[:], data16[:], sidx16[:],
                                    channels=P, num_elems=K1p, num_idxs=CAND)
            stg = spool.tile([P * K1p], i16, tag="stg")
            nc.sync.dma_start(stg[:].rearrange("(p c) -> p c", p=P), tc_i[:])
            idx16 = smp.tile([P, NC16p], i16, tag="idx16")
            srcv = stg[:].rearrange("(g p k) -> p k g", g=8, p=16)
            nc.sync.dma_start(idx16[0:16, :], srcv)
            try:
                nc.sync.dma_start(
                    idx16[16:, :].rearrange("(r p) c -> r p c", r=7),
                    idx16[0:16, :].broadcast_to((7, 16, NC16p)))
            except Exception:
                for rr in range(1, 8):
                    nc.sync.dma_start(idx16[16 * rr:16 * (rr + 1), :], idx16[0:16, :])

            # --- risky-row flags: (a) possible chunk overflow, (b) tiny
            #     rank-33/34 margin, (c) mark-count != K1.  Host recomputes. ---
            flg = smp.tile([P, 1], f32, tag="flg")
            f40 = smp.tile([P, SELC], f32, tag="f40")
            v8l = v8[:].rearrange("p (c e) -> p c e", e=8)[:, :, 7]
            nc.gpsimd.tensor_scalar(f40[:], v8l, m5[:, 0:1], None, op0=ALU.is_ge)
            nc.vector.tensor_reduce(flg[:], f40[:], axis=AX.X, op=ALU.max)
            fm = smp.tile([P, 1], f32, tag="fm")
            nc.gpsimd.tensor_sub(fm[:], m5[:, 0:1], m5[:, 1:2])
            nc.gpsimd.tensor_scalar(fm[:], fm[:], 5.0e-4, None, op0=ALU.is_lt)
            nc.gpsimd.tensor_add(flg[:], flg[:], fm[:])
            fc = smp.tile([P, 1], f32, tag="fc")
            nc.gpsimd.tensor_scalar(fc[:], fcnt[:], float(K1), None, op0=ALU.subtract)
            nc.gpsimd.tensor_scalar(fc[:], fc[:], 0.0, None, op0=ALU.not_equal)
            nc.gpsimd.tensor_add(flg[:], flg[:], fc[:])
            nc.sync.dma_start(flg_d.ap()[ts_:ts_ + P, :], flg[:])


            d.update(ts=ts_, idx16=idx16)
            return d

        def gather_part(d, last=False):
            # gathers + z-accum: g rows of (h_l | 0.25 p), then
            # g[:, :, :257] += (h_r | 0.25 q) via DMA-engine accumulate
            g = gp.tile([P, K1p, GROW], f16, tag="g")
            hrq, idx16 = d["hrq"], d["idx16"]
            for c0, c1 in ((0, 8), (8, 16), (16, 24), (24, 32), (32, K1p)):
                ni = (c1 - c0) * P
                nc.gpsimd.dma_gather(g[:, c0:c1, :], hl_d[:],
                                     idx16[:, c0 * 8:c1 * 8],
                                     num_idxs=ni, num_idxs_reg=ni,
                                     elem_size=GROW)
                if not last:
                    ca, cb = c0, min(c1, K1)
                    nc.gpsimd.dma_start(
                        g[:, ca:cb, :DOUT + 1],
                        hrq[:].rearrange("p (o d) -> p o d", o=1)
                        .broadcast_to((P, cb - ca, DOUT + 1)),
                        accum_op=ALU.add)
            u = up.tile([P, K1, DOUT], f16, tag="u")
            d.update(g=g, u=u)

        def relu_part(d, last=False):
            u, g, hrq = d["u"], d["g"], d["hrq"]
            if last:
                # z into u on DVE, then relu u -> g's score region
                for c0, c1 in ((0, 8), (8, 16), (16, 24), (24, 32), (32, K1)):
                    nc.vector.tensor_add(
                        u[:, c0:c1, :], g[:, c0:c1, :DOUT],
                        hrq[:, :DOUT].rearrange("p (o d) -> p o d", o=1)
                        .broadcast_to((P, c1 - c0, DOUT)))
                nc.vector.tensor_scalar_max(g[:, :K1, :DOUT], u[:], 0.0)
            else:
                nc.scalar.activation(u[:], g[:, :K1, :DOUT], FT.Relu)

        def score_part(d):
            g, u, hrq, ts_ = d["g"], d["u"], d["hrq"], d["ts"]
            last = bool(d.get("last"))
            rt = g[:, :K1, :DOUT] if last else u[:]     # relu'd tensor
            zt = u[:] if last else g[:, :K1, :DOUT]     # z tensor (ws source)
            wt = g[:, :K1, :DOUT] if last else u[:]     # ws scratch

            def tree(off, w, eng):
                while w > 1:
                    a_ = w // 2
                    nw = w - a_
                    eng.tensor_add(rt[:, :, off:off + a_],
                                   rt[:, :, off:off + a_],
                                   rt[:, :, off + nw:off + w])
                    w = nw

            ee = smp.tile([P, K1], f16, tag="ee")
            if 0 < dpos < DOUT:
                tree(0, dpos, nc.vector if last else nc.gpsimd)
                tree(dpos, dneg, nc.vector)
                nc.vector.tensor_sub(ee[:], rt[:, :, 0], rt[:, :, dpos])
            else:
                tree(0, DOUT, nc.vector)
                if dpos == 0:
                    nc.vector.tensor_scalar_mul(ee[:], rt[:, :, 0], -1.0)
                else:
                    nc.vector.tensor_copy(ee[:], rt[:, :, 0])
            # e/0.8 = e8 + 0.25*p + 0.25*q;  softmax via Act exp, scale=0.8
            nc.vector.tensor_add(ee[:], ee[:], g[:, :K1, DOUT])
            if last:
                nc.vector.tensor_scalar_add(ee[:], ee[:], d["q02"][:])
            mx = smp.tile([P, 1], f32, tag="mx")
            nc.vector.reduce_max(mx[:], ee[:], axis=AX.X)
            nc.vector.tensor_scalar_mul(mx[:], mx[:], -0.8)
            ex = smp.tile([P, K1], f32, tag="ex")
            nc.scalar.activation(ex[:], ee[:], FT.Exp, bias=mx[:], scale=0.8)
            sm = smp.tile([P, 1], f32, tag="sm")
            nc.vector.reduce_sum(sm[:], ex[:], axis=AX.X)
            nc.vector.reciprocal(sm[:], sm[:])
            al = smp.tile([P, K1], f32, tag="al")
            nc.vector.tensor_scalar_mul(al[:], ex[:], sm[:])

            # --- weighted sum: alpha-muls into ws scratch, k-tree ---
            for k in range(K1):
                nc.vector.tensor_scalar_mul(wt[:, k, :], zt[:, k, :],
                                            al[:, k:k + 1])
            kw = K1
            while kw > 1:
                a_ = kw // 2
                nw = kw - a_
                nc.vector.tensor_add(wt[:, 0:a_, :], wt[:, 0:a_, :],
                                     wt[:, nw:kw, :])
                kw = nw
            ob = op.tile([P, DOUT], f32, tag="ob")
            nc.vector.tensor_sub(ob[:], wt[:, 0, :], hrq[:, :DOUT])
            nc.vector.tensor_mul(ob[:], ob[:], invar_s)
            nc.vector.tensor_add(ob[:], ob[:], biasr_s)
            nc.sync.dma_start(out_d.ap()[ts_:ts_ + P, :], ob[:])

        prev = None
        for ii, ts_ in enumerate(starts):
            _cb = (lambda p=prev: relu_part(p)) if prev is not None else None
            last_t = (ii == len(starts) - 1)
            d = sel_part(ts_, relu_cb=_cb, last=last_t)
            gather_part(d, last=last_t)
            if ii == 0:
                phase_b()
            if prev is not None:
                score_part(prev)
            prev = d
        prev["last"] = True
        relu_part(prev, last=True)
        score_part(prev)

    nc.compile()
    return nc


def host_prep(x, W_l, b_l, W_r, b_r, att, bias, cfg):
    """Build the per-core input maps (numpy only; cheap O(N*D) work)."""
    N, DOUT = cfg["N"], cfg["DOUT"]
    ROWS = N // cfg["NCORES"]
    SELC = N // cfg["SELW"]
    CAND = SELC * 8

    x = np.asarray(x, np.float32)
    xh, xl = _split16(x)
    xhT = np.ascontiguousarray(xh.T)
    xlT = np.ascontiguousarray(xl.T)
    sq = (x.astype(np.float64) ** 2).sum(1)
    sv = (-0.5 * sq).astype(np.float32)
    shi = sv.astype(F16)
    slo = (sv - shi.astype(np.float32)).astype(F16)
    seed2 = np.stack([shi, slo], 0)
    seed2o = np.concatenate([seed2, np.ones((2, P), F16)], 1)

    att = np.asarray(att, np.float32)
    # permute output features: att>0 block first; fold |att| into weights.
    perm = np.argsort(att <= 0, kind="stable")
    aperm = att[perm]
    aabs = np.maximum(np.abs(aperm), 1e-30)
    wp = (W_l.astype(np.float64) @ att.astype(np.float64)).astype(np.float32)
    wle = np.concatenate([W_l[:, perm] * aabs[None, :], 0.25 * wp[:, None]],
                         1).astype(F16)
    wr = (np.asarray(W_r, np.float32)[:, perm] * aabs[None, :]).astype(F16)
    bl = np.asarray(b_l, np.float32)
    # z-bias: (b_l+b_r) |att|-scaled rides on h_r; p-column omits att.b_l but
    # q (= att.h_r incl. b_l+b_r) supplies it, so p_j + q_n is exact.
    brr = np.tile(((np.asarray(b_r, np.float32) + bl)[perm]
                   * aabs)[None, :], (P, 1))
    sgnr = np.tile(np.sign(aperm)[None, :], (P, 1)).astype(np.float32)
    invar = np.tile((1.0 / aabs)[None, :], (P, 1)).astype(np.float32)
    biasr = np.tile((np.asarray(bias, np.float32)
                     + np.asarray(b_l, np.float32))[perm][None, :], (P, 1))
    cwrow = (np.arange(CAND) // 8 * cfg["SELW"] + 1).astype(np.float32)
    cw1 = np.tile(cwrow[None, :], (P, 1))
    w16 = np.concatenate([wle, wr], 1)
    f32c = np.concatenate([brr, sgnr, invar, biasr, cw1], 1).astype(np.float32)

    shared = dict(xhT=xhT, xlT=xlT, seed2o=seed2o, w16=w16, f32c=f32c)
    host_prep.last_perm = perm
    host_prep.last_dpos = int((aperm > 0).sum())
    in_maps = []
    for c in range(cfg["NCORES"]):
        R = c * ROWS
        m = dict(shared)
        m["xhTo"] = np.ascontiguousarray(xhT[:, R:R + ROWS])
        m["xlTo"] = np.ascontiguousarray(xlT[:, R:R + ROWS])
        in_maps.append(m)
    return in_maps


_PROG_CACHE = {}


def _get_program(dpos):
    if dpos not in _PROG_CACHE:
        cfg = dict(CFG)
        cfg["_dpos"] = dpos
        _PROG_CACHE[dpos] = build_program(cfg)
    return _PROG_CACHE[dpos]


def kernel(x, W_l, b_l, W_r, b_r, att, bias, _trace=False):
    from concourse import bass_utils

    cfg = CFG
    in_maps = host_prep(x, W_l, b_l, W_r, b_r, att, bias, cfg)
    perm = host_prep.last_perm
    nc = _get_program(host_prep.last_dpos)
    try:
        res = bass_utils.run_bass_kernel_spmd(
            nc, in_maps, core_ids=list(range(cfg["NCORES"])), trace=_trace)
    except ModuleNotFoundError:
        res = bass_utils.run_bass_kernel_spmd(
            nc, in_maps, core_ids=list(range(cfg["NCORES"])), trace=False)
    outp = np.concatenate([r["out"] for r in res.results], 0)
    out = np.empty_like(outp)
    out[:, perm] = outp
    kernel.last_exec_time_ns = res.exec_time_ns
    flags = np.concatenate([r["flags"][:, 0] for r in res.results], 0)
    rows = np.where(flags != 0.0)[0]
    if rows.size:
        _patch_rows(out, rows, x, W_l, b_l, W_r, b_r, att, bias, cfg)
    return out.astype(np.float32)


def _patch_rows(out, rows, x, W_l, b_l, W_r, b_r, att, bias, cfg):
    """Exact (float64) recompute of flagged rows (near-ties / rare overflow),
    vectorized over the flagged set."""
    K = cfg["KNN"]
    x64 = np.asarray(x, np.float64)
    sq = (x64 * x64).sum(1)
    h_l = x64 @ np.asarray(W_l, np.float64) + np.asarray(b_l, np.float64)
    att64 = np.asarray(att, np.float64)
    W_r64 = np.asarray(W_r, np.float64)
    rows = np.asarray(rows)
    R = len(rows)
    d = sq[None, :] + sq[rows, None] - 2.0 * (x64[rows] @ x64.T)  # [R, N]
    d[np.arange(R), rows] = np.inf
    nbr = np.argpartition(d, K, axis=1)[:, :K]                    # [R, K]
    src = np.concatenate([nbr, rows[:, None]], 1)                 # [R, K+1]
    h_r = x64[rows] @ W_r64 + np.asarray(b_r, np.float64)         # [R, D]
    z = h_l[src] + h_r[:, None, :]                                # [R, K+1, D]
    lr = np.where(z > 0, z, 0.2 * z)
    e = lr @ att64                                                # [R, K+1]
    e = e - e.max(1, keepdims=True)
    a = np.exp(e)
    a /= a.sum(1, keepdims=True)
    o = np.einsum('rk,rkd->rd', a, h_l[src]) + np.asarray(bias, np.float64)
    out[rows] = o.astype(np.float32)
